# revision 5
# baseline (speedup 1.0000x reference)
"""Multi-head causal attention (B=2, S=2048, D=2048, 16 heads) on 8 TRN2 cores.

Sharding: 2-way batch parallel x 4-way head tensor-parallel (4 heads/core).
Each core computes q/k/v projections for its 4 heads, causal softmax
attention, and a partial o-projection; the host sums the 4 partials per batch.

v5: q/k/v projections in fp8-e4m3 DoubleRow matmuls (0.5 cyc/col on the PE,
~3x fp16 measured) with 3-term hi/lo compensation:
    x ~ xh + xl,  32*w ~ wh + wl   (w pre-scaled x32 on host: raw w entries
    sit in e4m3's subnormal range and their residuals flush to zero)
    x@w*32 ~ xh@wh + xh@wl + xl@wh     (xl@wl dropped, ~0.05% error)
24 DoubleRow matmuls (3 products x 8 dt-pairs) per 512-wide output tile
instead of 16 f32r matmuls: 0.75x PE cycles and 0.5x input DMA bytes (the
kernel is partly DMA-bound: ~110-130 GB/s effective per-core HBM bandwidth
with 8 cores streaming).  The x32 scale rides through q/k (scores x1024,
folded into the exp scale) and v (attT x32, folded into wo/32 on host).

Phase 2 (fp16 attention, unchanged from v4): per (i-chunk, head): score
pairs kT@qT -> exp (scale fused, per-half offsets so only valid causal
columns are computed), triangular mask on diagonal 128-blocks; PV matmuls
one pair behind the exps; denominator folded on DVE then one ones[128,128]
matmul + DVE reciprocal; normalize into attT.  After 4 heads: fused
o-projection of the chunk's 4 s-tiles -> fp16 partial out rows DMA'd
immediately.
"""

import math

import numpy as np

B, S, D = 2, 2048, 2048
HEADS, HEAD_DIM = 16, 128
P = 128
JC = 512          # per-core projection width (4 heads x 128)
SC = 512          # s-chunk / matmul moving width
DT = D // P       # 16 contraction tiles
NSC = S // SC     # 4 s-chunks
NST = S // P      # 16 s-tiles
HPC = 4           # heads per core
N_CORES = 8
WSCALE = 32.0     # host pre-scale on wq/wk/wv before fp8 split
SCALE = (1.0 / math.sqrt(HEAD_DIM)) / (WSCALE * WSCALE)

_NC_CACHE = {}


def build_module(reps=1, phases=(1, 2, 3)):
    """Build + compile the (single-program SPMD) Bass module once."""
    phases = tuple(phases)
    key = (reps, phases)
    if key in _NC_CACHE:
        return _NC_CACHE[key]

    from contextlib import ExitStack

    import concourse.tile as tile
    from concourse import bacc
    import concourse.mybir as mybir

    f16 = mybir.dt.float16
    f32 = mybir.dt.float32
    f8 = mybir.dt.float8e4
    DR = mybir.MatmulPerfMode.DoubleRow
    FT = mybir.ActivationFunctionType

    nc = bacc.Bacc(
        "TRN2", target_bir_lowering=False, debug=False, num_devices=N_CORES
    )

    # Host pre-swizzled partition-major layouts (one fat contiguous DRAM run
    # per SBUF partition per DMA).  hi/lo fp8 pairs for x and the projection
    # weights; wo stays fp16.
    xhh = nc.dram_tensor("xhh", [P, NSC, DT, SC], f8, kind="ExternalInput").ap()
    xhl = nc.dram_tensor("xhl", [P, NSC, DT, SC], f8, kind="ExternalInput").ap()
    wps = {}
    for w in ("wq", "wk", "wv"):
        for part in ("h", "l"):
            wps[w + part] = nc.dram_tensor(
                w + part, [P, DT, JC], f8, kind="ExternalInput"
            ).ap()
    woh = nc.dram_tensor("woh", [P, HPC, D], f16, kind="ExternalInput").ap()
    # mask[j, c] = 1 iff j <= c : causal triangle for a diagonal 128-block
    mask = nc.dram_tensor("mask", [P, P], f16, kind="ExternalInput").ap()
    ones = nc.dram_tensor("ones", [P, P], f16, kind="ExternalInput").ap()
    out = nc.dram_tensor("out", [S, D], f16, kind="ExternalOutput").ap()

    with tile.TileContext(nc) as tc, ExitStack() as ctx:
        consts = ctx.enter_context(tc.tile_pool(name="consts", bufs=1))

        mask_sb = consts.tile([P, P], f16, tag="mask", name="mask_sb")
        nc.sync.dma_start(mask_sb, mask)
        ones_sb = consts.tile([P, P], f16, tag="ones", name="ones_sb")
        nc.sync.dma_start(ones_sb, ones)

        for _rep in range(reps):
            with ExitStack() as prep:
                # resident fp16 activations: 3 x 2.1MB
                kvpool = prep.enter_context(tc.tile_pool(name="kvpool", bufs=1))
                kT_all = kvpool.tile([P, HPC, S], f16, tag="kT", name="kT_all")
                qT_all = kvpool.tile([P, HPC, S], f16, tag="qT", name="qT_all")
                v_all = kvpool.tile([P, NST, JC], f16, tag="v", name="v_all")

                # ---------- Phase 1: q/k/v projections (fp8 DoubleRow) ----
                with ExitStack() as p1:
                    wpool = p1.enter_context(tc.tile_pool(name="wpool", bufs=1))
                    xpool = p1.enter_context(tc.tile_pool(name="xpool", bufs=2))
                    psum1 = p1.enter_context(
                        tc.tile_pool(name="psum1", bufs=2, space="PSUM")
                    )

                    # all six fp8 weight tensors resident (6 x 1.05MB).
                    # wv hi first (first consumer), in 2 pieces so the first
                    # v chain starts after ~0.8MB of DMA.
                    w_sb = {}
                    for wn in ("wvh", "wvl", "wkh", "wqh", "wkl", "wql"):
                        w_sb[wn] = wpool.tile(
                            [P, DT, JC], f8, tag=wn, name=wn + "_sb"
                        )
                    for half in range(2):
                        dts = slice(half * 8, (half + 1) * 8)
                        nc.scalar.dma_start(
                            w_sb["wvh"][:, dts, :], wps["wvh"][:, dts, :]
                        )
                    nc.scalar.dma_start(w_sb["wvl"], wps["wvl"])

                    for sc in range(NSC):
                        xch = xpool.tile(
                            [P, DT, SC], f8, tag="xh", name=f"xch_{sc}"
                        )
                        xcl = xpool.tile(
                            [P, DT, SC], f8, tag="xl", name=f"xcl_{sc}"
                        )
                        np_x = 4 if sc == 0 else 2
                        for piece in range(np_x):
                            w8 = DT // np_x
                            dts = slice(piece * w8, (piece + 1) * w8)
                            nc.sync.dma_start(xch[:, dts, :], xhh[:, sc, dts, :])
                        for piece in range(2):
                            dts = slice(piece * 8, (piece + 1) * 8)
                            nc.sync.dma_start(xcl[:, dts, :], xhl[:, sc, dts, :])

                        # v projection: x stationary, wv moving.
                        # products: xh*wvh, xh*wvl, xl*wvh (24 DR matmuls
                        # per psum tile, t-interleaved across 4 banks)
                        ps_v = [
                            psum1.tile(
                                [P, JC], f32, tag=f"pv{t}", bufs=1, name="ps_v"
                            )
                            for t in range(4)
                        ]
                        vprods = [(xch, "wvh"), (xcl, "wvh"), (xch, "wvl")]
                        for pi, (xc_p, wn) in enumerate(vprods):
                            for g in range(DT // 2):
                                gs = slice(2 * g, 2 * g + 2)
                                for t in range(4):
                                    nc.tensor.matmul(
                                        ps_v[t],
                                        lhsT=xc_p[:, gs, t * P:(t + 1) * P],
                                        rhs=w_sb[wn][:, gs, :],
                                        start=(pi == 0 and g == 0),
                                        stop=(pi == 2 and g == DT // 2 - 1),
                                        perf_mode=DR,
                                    )
                        for t in range(4):
                            nc.vector.tensor_copy(
                                v_all[:, sc * 4 + t, :], ps_v[t]
                            )
                        if sc == 0:
                            # queue the remaining resident weights (k first:
                            # it's the next consumer)
                            nc.scalar.dma_start(w_sb["wkh"], wps["wkh"])
                            nc.scalar.dma_start(w_sb["wqh"], wps["wqh"])
                            nc.scalar.dma_start(w_sb["wkl"], wps["wkl"])
                            nc.scalar.dma_start(w_sb["wql"], wps["wql"])

                        # k / q projections: w stationary, x moving
                        for wn, dst, copy_eng in (
                            ("wk", kT_all, nc.scalar.copy),
                            ("wq", qT_all, None),
                        ):
                            for t in range(4):
                                ps = psum1.tile(
                                    [P, SC], f32, tag="pj", name=f"ps_{wn}"
                                )
                                prods = [
                                    (wn + "h", xch),
                                    (wn + "l", xch),
                                    (wn + "h", xcl),
                                ]
                                for pi, (wpn, xc_p) in enumerate(prods):
                                    for g in range(DT // 2):
                                        gs = slice(2 * g, 2 * g + 2)
                                        nc.tensor.matmul(
                                            ps,
                                            lhsT=w_sb[wpn][
                                                :, gs, t * P:(t + 1) * P
                                            ],
                                            rhs=xc_p[:, gs, :],
                                            start=(pi == 0 and g == 0),
                                            stop=(
                                                pi == 2 and g == DT // 2 - 1
                                            ),
                                            perf_mode=DR,
                                        )
                                if copy_eng is not None:
                                    copy_eng(
                                        dst[:, t, sc * SC:(sc + 1) * SC], ps
                                    )
                                else:
                                    nc.vector.tensor_copy(
                                        dst[:, t, sc * SC:(sc + 1) * SC], ps
                                    )

                # ---------- Phase 2: attention + fused o-proj ----------
                if 2 not in phases:
                    continue
                with ExitStack() as p2:
                    opool = p2.enter_context(tc.tile_pool(name="opool", bufs=1))
                    attp = p2.enter_context(tc.tile_pool(name="attp", bufs=2))
                    ppool = p2.enter_context(tc.tile_pool(name="ppool", bufs=5))
                    fpool = p2.enter_context(tc.tile_pool(name="fpool", bufs=2))
                    rpool = p2.enter_context(tc.tile_pool(name="rpool", bufs=2))
                    ostage = p2.enter_context(
                        tc.tile_pool(name="ostage", bufs=2)
                    )
                    psum2 = p2.enter_context(
                        tc.tile_pool(name="psum2", bufs=2, space="PSUM")
                    )

                    # o-proj weights fp16 (2.1MB), one fat DMA at phase-2
                    # entry; first needed ~4 heads later so fully hidden
                    woTs = opool.tile([P, HPC, D], f16, tag="wo", name="woTs")
                    nc.scalar.dma_start(woTs, woh)

                    for ic in range(NSC):
                        njt = 4 * ic + 4  # causal: j-tiles 0..njt-1
                        offs = [
                            max(0, (jt - 4 * ic) * P) for jt in range(njt)
                        ]
                        attT_c = attp.tile(
                            [P, HPC, SC], f16, tag="attT", name=f"attT_{ic}"
                        )
                        for h in range(HPC):
                            # scores in pairs of j-tiles -> one wide exp;
                            # PV matmuls run one pair behind the exps
                            facc = fpool.tile(
                                [P, SC], f16, tag="facc", name="facc"
                            )
                            ps_pv = psum2.tile(
                                [P, SC], f32, tag="pv", name="ps_pv"
                            )
                            pts = []  # (wide prob tile, half index u) per jt
                            for g in range(njt // 2):
                                ps_s = psum2.tile(
                                    [P, 2 * SC], f32, tag="score", bufs=2,
                                    name="ps_s",
                                )
                                ptw = ppool.tile(
                                    [P, 2 * SC], f16, tag="prob", name="pt"
                                )
                                o0, o1 = offs[2 * g], offs[2 * g + 1]
                                for u, off in ((0, o0), (1, o1)):
                                    jt = 2 * g + u
                                    pts.append((ptw, u))
                                    nc.tensor.matmul(
                                        ps_s[:, u * SC + off:(u + 1) * SC],
                                        lhsT=kT_all[
                                            :, h, jt * P:(jt + 1) * P
                                        ],
                                        rhs=qT_all[
                                            :, h,
                                            ic * SC + off:(ic + 1) * SC,
                                        ],
                                        start=True,
                                        stop=True,
                                    )
                                if o0 == o1:
                                    nc.scalar.activation(
                                        ptw[:, o0:], ps_s[:, o0:],
                                        FT.Exp, scale=SCALE,
                                    )
                                else:
                                    nc.scalar.activation(
                                        ptw[:, o0:SC], ps_s[:, o0:SC],
                                        FT.Exp, scale=SCALE,
                                    )
                                    nc.scalar.activation(
                                        ptw[:, SC + o1:],
                                        ps_s[:, SC + o1:],
                                        FT.Exp, scale=SCALE,
                                    )
                                for u, off in ((0, o0), (1, o1)):
                                    jt = 2 * g + u
                                    if jt >= 4 * ic:
                                        # triangular mask on diagonal block
                                        nc.vector.tensor_mul(
                                            out=ptw[
                                                :,
                                                u * SC + off:u * SC + off + P,
                                            ],
                                            in0=ptw[
                                                :,
                                                u * SC + off:u * SC + off + P,
                                            ],
                                            in1=mask_sb,
                                        )
                                # denominator fold on DVE (fp16, all-SBUF)
                                for u, off in ((0, o0), (1, o1)):
                                    src = ptw[:, u * SC + off:(u + 1) * SC]
                                    if g == 0 and u == 0:
                                        nc.vector.tensor_copy(facc, src)
                                    else:
                                        nc.vector.tensor_add(
                                            facc[:, off:], facc[:, off:], src
                                        )
                                # PV two pairs behind (slack for the exp
                                # latency on the ScalarE so the PE never
                                # waits on a fresh activation)
                                if g > 1:
                                    for jt in (2 * g - 4, 2 * g - 3):
                                        off = offs[jt]
                                        pw, u = pts[jt]
                                        nc.tensor.matmul(
                                            ps_pv[:, off:],
                                            lhsT=v_all[
                                                :, jt,
                                                h * HEAD_DIM:
                                                (h + 1) * HEAD_DIM,
                                            ],
                                            rhs=pw[
                                                :, u * SC + off:(u + 1) * SC
                                            ],
                                            start=(jt == 0),
                                            stop=False,
                                            skip_group_check=True,
                                        )
                            for jt in range(max(0, njt - 4), njt):
                                off = offs[jt]
                                pw, u = pts[jt]
                                nc.tensor.matmul(
                                    ps_pv[:, off:],
                                    lhsT=v_all[
                                        :, jt, h * HEAD_DIM:(h + 1) * HEAD_DIM
                                    ],
                                    rhs=pw[:, u * SC + off:(u + 1) * SC],
                                    start=(jt == 0),
                                    stop=(jt == njt - 1),
                                    skip_group_check=True,
                                )
                            # denominator: one matmul per (chunk, head)
                            ps_den = psum2.tile(
                                [P, SC], f32, tag="den", bufs=1, name="ps_den"
                            )
                            nc.tensor.matmul(
                                ps_den, lhsT=ones_sb, rhs=facc,
                                start=True, stop=True,
                            )
                            rec = rpool.tile(
                                [P, SC], f32, tag="rec", name="rec"
                            )
                            nc.vector.reciprocal(rec, ps_den)
                            nc.vector.tensor_mul(
                                out=attT_c[:, h, :], in0=ps_pv, in1=rec
                            )

                        # fused o-proj for this chunk's 4 s-tiles
                        if 3 not in phases:
                            continue
                        for t in range(4):
                            st = 4 * ic + t
                            og = ostage.tile([P, D], f16, tag="og", name="og")
                            for mc in range(D // SC):
                                # alternate den/po banks so each og copy
                                # hides behind the other bank's matmuls
                                ps_o = psum2.tile(
                                    [P, SC], f32,
                                    tag=("den" if mc % 2 == 0 else "po"),
                                    bufs=1, name="ps_o",
                                )
                                for hh in range(HPC):
                                    nc.tensor.matmul(
                                        ps_o,
                                        lhsT=attT_c[:, hh, t * P:(t + 1) * P],
                                        rhs=woTs[:, hh, mc * SC:(mc + 1) * SC],
                                        start=(hh == 0),
                                        stop=(hh == HPC - 1),
                                    )
                                # split og copies DVE/ScalarE: DVE is the
                                # busiest non-PE engine in phase 2
                                if mc % 2 == 0:
                                    nc.vector.tensor_copy(
                                        og[:, mc * SC:(mc + 1) * SC], ps_o
                                    )
                                else:
                                    nc.scalar.copy(
                                        og[:, mc * SC:(mc + 1) * SC], ps_o
                                    )
                            nc.sync.dma_start(
                                out[st * P:(st + 1) * P, :], og
                            )

    nc.compile()
    _NC_CACHE[key] = nc
    return nc


def _split8(a):
    import ml_dtypes

    hi = a.astype(ml_dtypes.float8_e4m3)
    lo = (a - hi.astype(np.float32)).astype(ml_dtypes.float8_e4m3)
    return np.ascontiguousarray(hi), np.ascontiguousarray(lo)


def make_in_maps(x, wq, wk, wv, wo):
    x = np.asarray(x, dtype=np.float32)
    wq = np.asarray(wq, dtype=np.float32)
    wk = np.asarray(wk, dtype=np.float32)
    wv = np.asarray(wv, dtype=np.float32)
    wo = np.asarray(wo, dtype=np.float32)
    # mask[j, c] = 1 iff key j visible to query c within a diagonal block
    causal = np.triu(np.ones((P, P), dtype=np.float16))
    ones = np.ones((P, P), dtype=np.float16)
    in_maps = []
    xsplit = {}
    for c in range(N_CORES):
        b, g = divmod(c, HPC)
        j0 = g * JC
        # partition-major fat layouts:
        #   xh [p, sc, dt, s] = x[b].T[(dt p), (sc s)], fp8 hi/lo
        if b not in xsplit:
            xt = np.ascontiguousarray(
                x[b].T.reshape(DT, P, NSC, SC).transpose(1, 2, 0, 3)
            )
            xsplit[b] = _split8(xt)
        xhh, xhl = xsplit[b]
        im = {"xhh": xhh, "xhl": xhl, "mask": causal, "ones": ones}
        #   w*h/l [p, dt, j] = (32*w[j0:j0+JC]).T[(dt p), j] fp8 hi/lo
        for wn, w in (("wq", wq), ("wk", wk), ("wv", wv)):
            wt = np.ascontiguousarray(
                (WSCALE * w[j0:j0 + JC]).T.reshape(DT, P, JC).transpose(1, 0, 2)
            )
            im[wn + "h"], im[wn + "l"] = _split8(wt)
        #   woh [p, hh, m] = (wo/32)[:, j0:j0+JC].T[(hh p), m]
        im["woh"] = np.ascontiguousarray(
            (wo[:, j0:j0 + JC] / WSCALE).T.reshape(HPC, P, D).transpose(1, 0, 2)
        ).astype(np.float16)
        in_maps.append(im)
    return in_maps


def combine_outputs(results):
    out = np.zeros((B, S, D), dtype=np.float32)
    for c in range(N_CORES):
        out[c // HPC] += np.asarray(results[c]["out"], dtype=np.float32)
    return out


def kernel(x, wq, wk, wv, wo):
    from concourse.bass_utils import run_bass_kernel_spmd

    nc = build_module()
    in_maps = make_in_maps(x, wq, wk, wv, wo)
    res = run_bass_kernel_spmd(nc, in_maps, list(range(N_CORES)))
    return combine_outputs(res.results)


# revision 14
# speedup vs baseline: 1.3596x; 1.3596x over previous
"""v11: v4 + o-proj of chunk ic-1 interleaved per-head into chunk ic (PE filler)."""

import math

import numpy as np

B, S, D = 2, 2048, 2048
HEADS, HEAD_DIM = 16, 128
P = 128
JC = 512          # per-core projection width (4 heads x 128)
SC = 512          # s-chunk / matmul moving width
DT = D // P       # 16 contraction tiles
NSC = S // SC     # 4 s-chunks
NST = S // P      # 16 s-tiles
HPC = 4           # heads per core
N_CORES = 8
SCALE = 1.0 / math.sqrt(HEAD_DIM)

_NC_CACHE = {}


def build_module(reps=1, phases=(1, 2, 3)):
    phases = tuple(phases)
    key = (reps, phases)
    if key in _NC_CACHE:
        return _NC_CACHE[key]

    from contextlib import ExitStack

    import concourse.tile as tile
    from concourse import bacc
    import concourse.mybir as mybir

    f16 = mybir.dt.float16
    f32 = mybir.dt.float32
    fr = mybir.dt.float32r
    FT = mybir.ActivationFunctionType

    nc = bacc.Bacc(
        "TRN2", target_bir_lowering=False, debug=False, num_devices=N_CORES
    )

    xh = nc.dram_tensor("xh", [P, NSC, DT, SC], fr, kind="ExternalInput").ap()
    wqh = nc.dram_tensor("wqh", [P, DT, JC], fr, kind="ExternalInput").ap()
    wkh = nc.dram_tensor("wkh", [P, DT, JC], fr, kind="ExternalInput").ap()
    wvh = nc.dram_tensor("wvh", [P, DT, JC], fr, kind="ExternalInput").ap()
    woh = nc.dram_tensor("woh", [P, HPC, D], f16, kind="ExternalInput").ap()
    mask = nc.dram_tensor("mask", [P, P], f16, kind="ExternalInput").ap()
    ones = nc.dram_tensor("ones", [P, P], f16, kind="ExternalInput").ap()
    out = nc.dram_tensor("out", [S, D], f16, kind="ExternalOutput").ap()

    with tile.TileContext(nc) as tc, ExitStack() as ctx:
        consts = ctx.enter_context(tc.tile_pool(name="consts", bufs=1))
        stage = ctx.enter_context(tc.tile_pool(name="stage", bufs=2))

        mask_sb = consts.tile([P, P], f16, tag="mask", name="mask_sb")
        nc.sync.dma_start(mask_sb, mask)
        ones_sb = consts.tile([P, P], f16, tag="ones", name="ones_sb")
        nc.sync.dma_start(ones_sb, ones)

        for _rep in range(reps):
            with ExitStack() as prep:
                kvpool = prep.enter_context(tc.tile_pool(name="kvpool", bufs=1))
                kT_all = kvpool.tile([P, HPC, S], f16, tag="kT", name="kT_all")
                qT_all = kvpool.tile([P, HPC, S], f16, tag="qT", name="qT_all")
                v_all = kvpool.tile([P, NST, JC], f16, tag="v", name="v_all")

                with ExitStack() as p1:
                    wpool = p1.enter_context(tc.tile_pool(name="wpool", bufs=1))
                    wvpool = p1.enter_context(
                        tc.tile_pool(name="wvpool", bufs=3)
                    )
                    xpool = p1.enter_context(tc.tile_pool(name="xpool", bufs=2))
                    psum1 = p1.enter_context(
                        tc.tile_pool(name="psum1", bufs=2, space="PSUM")
                    )

                    wq_sb = wpool.tile([P, DT, JC], fr, tag="wq", name="wq_sb")
                    wk_sb = wpool.tile([P, DT, JC], fr, tag="wk", name="wk_sb")

                    for sc in range(NSC):
                        xc = xpool.tile(
                            [P, DT, SC], fr, tag="x", name=f"xc_{sc}"
                        )
                        if sc == 0:
                            for q4 in range(4):
                                dts = slice(q4 * 4, (q4 + 1) * 4)
                                nc.sync.dma_start(
                                    xc[:, dts, :], xh[:, sc, dts, :]
                                )
                        else:
                            for q8 in range(2):
                                dts = slice(q8 * 8, (q8 + 1) * 8)
                                nc.sync.dma_start(
                                    xc[:, dts, :], xh[:, sc, dts, :]
                                )

                        ps_v = [
                            psum1.tile(
                                [P, JC], f32, tag=f"pv{t}", bufs=1, name="ps_v"
                            )
                            for t in range(4)
                        ]
                        wv_q4 = []
                        for q4 in range(4):
                            wv_p = wvpool.tile(
                                [P, 4, JC], fr, tag="wv", name="wv_p"
                            )
                            wv_q4.append(wv_p)
                            nc.scalar.dma_start(
                                wv_p, wvh[:, q4 * 4:(q4 + 1) * 4, :]
                            )
                        for dt in range(DT):
                            wv_dt = wv_q4[dt // 4][:, dt % 4, :]
                            for t in range(4):
                                nc.tensor.matmul(
                                    ps_v[t],
                                    lhsT=xc[:, dt, t * P:(t + 1) * P],
                                    rhs=wv_dt,
                                    start=(dt == 0),
                                    stop=(dt == DT - 1),
                                )
                        for t in range(4):
                            nc.vector.tensor_copy(
                                v_all[:, sc * 4 + t, :], ps_v[t]
                            )
                        if sc == 0:
                            nc.scalar.dma_start(wk_sb, wkh)
                            nc.scalar.dma_start(wq_sb, wqh)

                        for t in range(4):
                            ps = psum1.tile([P, SC], f32, tag="pj", name="ps_k")
                            for dt in range(DT):
                                nc.tensor.matmul(
                                    ps,
                                    lhsT=wk_sb[:, dt, t * P:(t + 1) * P],
                                    rhs=xc[:, dt, :],
                                    start=(dt == 0),
                                    stop=(dt == DT - 1),
                                )
                            nc.scalar.copy(
                                kT_all[:, t, sc * SC:(sc + 1) * SC], ps
                            )

                        for t in range(4):
                            ps = psum1.tile([P, SC], f32, tag="pj", name="ps_q")
                            for dt in range(DT):
                                nc.tensor.matmul(
                                    ps,
                                    lhsT=wq_sb[:, dt, t * P:(t + 1) * P],
                                    rhs=xc[:, dt, :],
                                    start=(dt == 0),
                                    stop=(dt == DT - 1),
                                )
                            nc.vector.tensor_copy(
                                qT_all[:, t, sc * SC:(sc + 1) * SC], ps
                            )

                if 2 not in phases:
                    continue
                with ExitStack() as p2:
                    opool = p2.enter_context(tc.tile_pool(name="opool", bufs=1))
                    attp = p2.enter_context(tc.tile_pool(name="attp", bufs=2))
                    ppool = p2.enter_context(tc.tile_pool(name="ppool", bufs=5))
                    fpool = p2.enter_context(tc.tile_pool(name="fpool", bufs=2))
                    rpool = p2.enter_context(tc.tile_pool(name="rpool", bufs=2))
                    ostage = p2.enter_context(
                        tc.tile_pool(name="ostage", bufs=2)
                    )
                    psum2 = p2.enter_context(
                        tc.tile_pool(name="psum2", bufs=2, space="PSUM")
                    )

                    woTs = opool.tile([P, HPC, D], f16, tag="wo", name="woTs")
                    nc.scalar.dma_start(woTs, woh)

                    def oproj_tile(attT_p, pic, t):
                        # o-projection tile for a COMPLETED chunk: its
                        # operands are ready, so these matmuls are
                        # dependency-free PE filler between heads of the
                        # current chunk (keeps the PE stream dense through
                        # the exp/fold waits and hides the o-proj)
                        st = 4 * pic + t
                        og = ostage.tile([P, D], f16, tag="og", name="og")
                        for mc in range(D // SC):
                            ps_o = psum2.tile(
                                [P, SC], f32, tag="po", bufs=2, name="ps_o"
                            )
                            for hh in range(HPC):
                                nc.tensor.matmul(
                                    ps_o,
                                    lhsT=attT_p[:, hh, t * P:(t + 1) * P],
                                    rhs=woTs[:, hh, mc * SC:(mc + 1) * SC],
                                    start=(hh == 0),
                                    stop=(hh == HPC - 1),
                                )
                            if mc % 2 == 0:
                                nc.vector.tensor_copy(
                                    og[:, mc * SC:(mc + 1) * SC], ps_o
                                )
                            else:
                                nc.scalar.copy(
                                    og[:, mc * SC:(mc + 1) * SC], ps_o
                                )
                        nc.sync.dma_start(out[st * P:(st + 1) * P, :], og)

                    attT_prev = None
                    for ic in range(NSC):
                        njt = 4 * ic + 4
                        offs = [
                            max(0, (jt - 4 * ic) * P) for jt in range(njt)
                        ]
                        attT_c = attp.tile(
                            [P, HPC, SC], f16, tag="attT", name=f"attT_{ic}"
                        )
                        for h in range(HPC):
                            facc = fpool.tile(
                                [P, SC], f16, tag="facc", name="facc"
                            )
                            ps_pv = psum2.tile(
                                [P, SC], f32, tag="pv", bufs=1, name="ps_pv"
                            )
                            pts = []
                            for g in range(njt // 2):
                                ps_s = psum2.tile(
                                    [P, 2 * SC], f32, tag="score", bufs=2,
                                    name="ps_s",
                                )
                                ptw = ppool.tile(
                                    [P, 2 * SC], f16, tag="prob", name="pt"
                                )
                                o0, o1 = offs[2 * g], offs[2 * g + 1]
                                for u, off in ((0, o0), (1, o1)):
                                    jt = 2 * g + u
                                    pts.append((ptw, u))
                                    nc.tensor.matmul(
                                        ps_s[:, u * SC + off:(u + 1) * SC],
                                        lhsT=kT_all[
                                            :, h, jt * P:(jt + 1) * P
                                        ],
                                        rhs=qT_all[
                                            :, h,
                                            ic * SC + off:(ic + 1) * SC,
                                        ],
                                        start=True,
                                        stop=True,
                                    )
                                if o0 == o1:
                                    nc.scalar.activation(
                                        ptw[:, o0:], ps_s[:, o0:],
                                        FT.Exp, scale=SCALE,
                                    )
                                else:
                                    nc.scalar.activation(
                                        ptw[:, o0:SC], ps_s[:, o0:SC],
                                        FT.Exp, scale=SCALE,
                                    )
                                    nc.scalar.activation(
                                        ptw[:, SC + o1:],
                                        ps_s[:, SC + o1:],
                                        FT.Exp, scale=SCALE,
                                    )
                                for u, off in ((0, o0), (1, o1)):
                                    jt = 2 * g + u
                                    if jt >= 4 * ic:
                                        nc.vector.tensor_mul(
                                            out=ptw[
                                                :,
                                                u * SC + off:u * SC + off + P,
                                            ],
                                            in0=ptw[
                                                :,
                                                u * SC + off:u * SC + off + P,
                                            ],
                                            in1=mask_sb,
                                        )
                                for u, off in ((0, o0), (1, o1)):
                                    src = ptw[:, u * SC + off:(u + 1) * SC]
                                    if g == 0 and u == 0:
                                        nc.vector.tensor_copy(facc, src)
                                    else:
                                        nc.vector.tensor_add(
                                            facc[:, off:], facc[:, off:], src
                                        )
                                if g > 1:
                                    for jt in (2 * g - 4, 2 * g - 3):
                                        off = offs[jt]
                                        pw, u = pts[jt]
                                        nc.tensor.matmul(
                                            ps_pv[:, off:],
                                            lhsT=v_all[
                                                :, jt,
                                                h * HEAD_DIM:
                                                (h + 1) * HEAD_DIM,
                                            ],
                                            rhs=pw[
                                                :, u * SC + off:(u + 1) * SC
                                            ],
                                            start=(jt == 0),
                                            stop=False,
                                            skip_group_check=True,
                                        )
                            for jt in range(max(0, njt - 4), njt):
                                off = offs[jt]
                                pw, u = pts[jt]
                                nc.tensor.matmul(
                                    ps_pv[:, off:],
                                    lhsT=v_all[
                                        :, jt, h * HEAD_DIM:(h + 1) * HEAD_DIM
                                    ],
                                    rhs=pw[:, u * SC + off:(u + 1) * SC],
                                    start=(jt == 0),
                                    stop=(jt == njt - 1),
                                    skip_group_check=True,
                                )
                            ps_den = psum2.tile(
                                [P, SC], f32, tag="den", bufs=1, name="ps_den"
                            )
                            nc.tensor.matmul(
                                ps_den, lhsT=ones_sb, rhs=facc,
                                start=True, stop=True,
                            )
                            rec = rpool.tile(
                                [P, SC], f32, tag="rec", name="rec"
                            )
                            nc.vector.reciprocal(rec, ps_den)
                            nc.vector.tensor_mul(
                                out=attT_c[:, h, :], in0=ps_pv, in1=rec
                            )
                            if 3 in phases and attT_prev is not None:
                                oproj_tile(attT_prev, ic - 1, h)
                        attT_prev = attT_c
                    if 3 in phases:
                        for t in range(4):
                            oproj_tile(attT_prev, NSC - 1, t)

    nc.compile()
    _NC_CACHE[key] = nc
    return nc


def make_in_maps(x, wq, wk, wv, wo):
    x = np.asarray(x, dtype=np.float32)
    wq = np.asarray(wq, dtype=np.float32)
    wk = np.asarray(wk, dtype=np.float32)
    wv = np.asarray(wv, dtype=np.float32)
    wo = np.asarray(wo, dtype=np.float32)
    causal = np.triu(np.ones((P, P), dtype=np.float16))
    ones = np.ones((P, P), dtype=np.float16)
    in_maps = []
    for c in range(N_CORES):
        b, g = divmod(c, HPC)
        j0 = g * JC
        xh = np.ascontiguousarray(
            x[b].T.reshape(DT, P, NSC, SC).transpose(1, 2, 0, 3)
        )
        wqh = np.ascontiguousarray(
            wq[j0:j0 + JC].T.reshape(DT, P, JC).transpose(1, 0, 2)
        )
        wkh = np.ascontiguousarray(
            wk[j0:j0 + JC].T.reshape(DT, P, JC).transpose(1, 0, 2)
        )
        wvh = np.ascontiguousarray(
            wv[j0:j0 + JC].T.reshape(DT, P, JC).transpose(1, 0, 2)
        )
        woh = np.ascontiguousarray(
            wo[:, j0:j0 + JC].T.reshape(HPC, P, D).transpose(1, 0, 2)
        ).astype(np.float16)
        in_maps.append(
            {
                "xh": xh,
                "wqh": wqh,
                "wkh": wkh,
                "wvh": wvh,
                "woh": woh,
                "mask": causal,
                "ones": ones,
            }
        )
    return in_maps


def combine_outputs(results):
    out = np.zeros((B, S, D), dtype=np.float32)
    for c in range(N_CORES):
        out[c // HPC] += np.asarray(results[c]["out"], dtype=np.float32)
    return out


def kernel(x, wq, wk, wv, wo):
    from concourse.bass_utils import run_bass_kernel_spmd

    nc = build_module()
    in_maps = make_in_maps(x, wq, wk, wv, wo)
    res = run_bass_kernel_spmd(nc, in_maps, list(range(N_CORES)))
    return combine_outputs(res.results)
